# revision 35
# baseline (speedup 1.0000x reference)
"""Trainium2 Bass kernel for nn_CelestialWaveAggregator.

Math: out[b,s,c] = tanh(h_c(agg[b,s,c])) where agg = wave_features @ M.T (M is
the per-body softmax aggregation matrix over ragged wave groups) and h_c is the
per-body 1->32->64->32->1 gelu MLP collapsed to a *univariate* function of the
aggregated scalar, approximated by a degree-9 polynomial in the bounded
Chebyshev product basis {1,t,u,ut,v,vt,vu,vut,w,wt} (u=T2, v=T4, w=T8).

Device strategy (8 cores, batch-sharded 16384 rows/core):
  - 117 partitions = 9 replica groups x 13 bodies; free dim 4x456 chunk cols.
  - Chunk-granular input: 4 contiguous ~927KB SWDGE loads ([113, 8208B]
    descriptors), issued in chunk order so chunk 0 lands ~1/4 into the load
    and compute pipelines behind the DMA.
  - PE: warmup junk matmuls during load (HAM un-throttle), then per pair of
    chunks: r-outer agg matmuls (shared lhsT between the two chunk halves)
    into a 2-bank PSUM pair tile; 7 poly terms x 2 halves of diag matmuls.
  - ACT: t copy w/ per-partition bias (PSUM->SBUF bf16), final tanh -> f32.
  - DVE: pair-wide (912-col) bf16 tower + products: 8 tensor_tensor +
    3 immediate tensor_scalar + 2 per-partition tensor_scalar.
  - Output accumulated in one [117, 1824] f32 SBUF tile; 2 big HWDGE stores
    (3648B/partition) overlap the second pair's compute / end the kernel.
Host: fit tables (aggregation matrix, per-body coefficients vs the bf16
device basis), pack inputs chunk-major, unshard output.
"""

import math
import os

import numpy as np

try:
    import ml_dtypes
    _BF16 = ml_dtypes.bfloat16
except Exception:  # pragma: no cover
    _BF16 = None

# ---- problem constants (hardcoded per contract) ----
LENS = np.array([9, 9, 9, 9, 9, 9, 9, 9, 9, 9, 12, 8, 3])
STARTS = np.concatenate([[5], 5 + np.cumsum(LENS)[:-1]])
MAXW, NW, NB = 12, 118, 13
B, S = 32, 4096
NCORES = 8
RPC = (B * S) // NCORES          # 16384 rows per core
NREP = 9                         # replica groups on partitions (9*13=117)
NP_USED = NREP * NB              # 117 used partitions
W0 = 5                           # first used wave channel
NWU = NW - W0                    # 113 used wave channels
NT = 4                           # chunks (2 pairs)
FW = 456                         # free columns per chunk
F = NT * FW                      # 1824 cols per replica (9*1824=16416 >= 16384)
NPAD = NREP * F - RPC            # 32 pad rows
NTERM = 7                        # poly diag terms: m1, m2, u, v, vu, w, wt

_f64 = np.float64


def _erf(x):
    try:
        from scipy.special import erf
        return erf(x)
    except Exception:
        return np.vectorize(math.erf)(x)


def _gelu(x):
    return 0.5 * x * (1.0 + _erf(x / np.sqrt(2.0)))


def _build_M(agg_logits):
    """Dense [13, 118] aggregation matrix from ragged softmax groups."""
    al = np.asarray(agg_logits, _f64)
    valid = np.arange(MAXW)[None, :] < LENS[:, None]
    logits = np.where(valid, al, -np.inf)
    w = np.exp(logits - logits.max(axis=-1, keepdims=True))
    w = w / w.sum(axis=-1, keepdims=True)
    w = np.where(valid, w, 0.0)
    M = np.zeros((NB, NW))
    idx = np.clip(STARTS[:, None] + np.arange(MAXW)[None, :], 0, NW - 1)
    for c in range(NB):
        for j in range(MAXW):
            M[c, idx[c, j]] += w[c, j]
    return M


def _h_fn(x, c, W1, b1, W2, b2, W3, b3, W4, b4):
    """Pre-tanh univariate MLP for body c, float64."""
    a = x[:, None] * W1[c, 0][None, :] + b1[c]
    h1 = _gelu(a)
    h2 = _gelu(h1 @ W2[c] + b2[c])
    h3 = _gelu(h2 @ W3[c] + b3[c])
    return h3 @ W4[c][:, 0] + b4[c, 0]


def _q(x):
    return x.astype(_BF16).astype(np.float32)


def _fit_tables(inputs):
    """Host precompute: aggregation matrix, per-body degree-9 Chebyshev
    product-basis coefficients (fit against the exact bf16 device basis)."""
    M = _build_M(inputs["agg_logits"])
    W = {k: np.asarray(inputs[k], _f64) for k in
         ("W1", "b1", "W2", "b2", "W3", "b3", "W4", "b4")}

    # calibration: per-body agg range from the actual data (+ margin)
    X = np.asarray(inputs["wave_features"], np.float32).reshape(-1, NW)
    agg = X.astype(_f64) @ M.T
    lo = agg.min(axis=0)
    hi = agg.max(axis=0)
    m = 0.12 * (hi - lo)
    lo, hi = lo - m, hi + m
    mid = 0.5 * (lo + hi)
    invhalf = 2.0 / (hi - lo)

    # fit c[13, 9] for basis [1, t, u, ut, v, vt, vu, vut, w]
    coefs = np.zeros((NB, 9))
    for c in range(NB):
        xs = np.linspace(lo[c], hi[c], 4001)
        hs = _h_fn(xs, c, **W)
        ys = np.tanh(hs)
        tg = _q(((xs - mid[c]) * invhalf[c]).astype(np.float32))
        t2 = _q(tg * tg)
        u = _q(t2 * 2 - 1)
        u2 = _q(u * u)
        v = _q(u2 * 2 - 1)
        v2 = _q(v * v)
        w = _q(v2 * 2 - 1)
        Bg = np.stack([np.ones_like(tg), tg, u, _q(u * tg), v, _q(v * tg),
                       _q(v * u), _q(_q(v * u) * tg), w], axis=1)
        wgt = 1.0 / np.cosh(hs) ** 2 + 1e-4
        for _ in range(10):
            sw = np.sqrt(wgt)
            coef, *_r = np.linalg.lstsq(Bg * sw[:, None], hs * sw, rcond=None)
            err = np.abs(np.tanh(Bg @ coef) - ys)
            wgt = wgt * (1.0 + 1.5 * err / (err.max() + 1e-12))
        coefs[c] = coef

    # wm: [128, 9*117 + 128] bf16 (channel rows padded to 128 so the SWDGE
    # engine swizzle sprays the load over all 16 engines and the matmuls get
    # FWL; extra 128 zero cols provide warmup weights).
    # Replica r's lhsT block col (r*13+c) = M[c]*invhalf[c].
    wm = np.zeros((128, NREP * NP_USED + 128), np.float32)
    Ms = (M[:, W0:] * invhalf[:, None]).T          # [113, 13]
    for r in range(NREP):
        for c in range(NB):
            wm[:NWU, r * NP_USED + r * NB + c] = Ms[:, c]

    # Poly is accumulated on PE as 8 diag-scaled terms (w=2v^2-1 folded:
    # c8*w = 2c8*v2 - c8, so a0' = c0 - c8):
    #   c1*t + c2*u + c3*ut + c4*v + c5*vt + c6*vu + c7*uvt + 2c8*v2
    # cst: [128, 8] fp32 (padded partitions): tbias, a0'
    cst = np.zeros((128, 8), np.float32)
    dvals = np.zeros((NP_USED, 8), np.float32)
    for r in range(NREP):
        for c in range(NB):
            p = r * NB + c
            cst[p, 0] = -mid[c] * invhalf[c]
            cst[p, 1] = coefs[c, 0] - coefs[c, 8]
            dvals[p] = [coefs[c, 1], coefs[c, 2], coefs[c, 3], coefs[c, 4],
                        coefs[c, 5], coefs[c, 6], coefs[c, 7],
                        2.0 * coefs[c, 8]]

    # diag lhsT stack [128, 8*117] bf16 (padded partitions) for terms
    # (t, u, ut, v, vt, vu, uvt, v2)
    dg = np.zeros((128, 8 * NP_USED), np.float32)
    for p in range(NP_USED):
        for k in range(8):
            dg[p, k * NP_USED + p] = dvals[p, k]
    return wm.astype(_BF16), dg.astype(_BF16), cst


_PROGRAM = None


def _build_program():
    """Build + compile the (SPMD, per-core) Bass/Tile program once."""
    global _PROGRAM
    if _PROGRAM is not None:
        return _PROGRAM

    from contextlib import ExitStack
    import concourse.bacc as bacc
    import concourse.tile as tile
    import concourse.mybir as mybir
    from concourse.tile import add_dep_helper
    from concourse._compat import axon_active

    f32 = mybir.dt.float32
    bf16 = mybir.dt.bfloat16
    Alu = mybir.AluOpType
    Act = mybir.ActivationFunctionType

    nc = bacc.Bacc(
        "TRN2",
        target_bir_lowering=False,
        debug=not axon_active(),
        enable_asserts=False,
        num_devices=NCORES,
    )
    xt = nc.dram_tensor("xt", [NT * 128, NREP * FW], bf16,
                        kind="ExternalInput").ap()
    wm = nc.dram_tensor("wm", [128, NREP * NP_USED + 128], bf16,
                        kind="ExternalInput").ap()
    dg = nc.dram_tensor("dg", [128, 8 * NP_USED], bf16,
                        kind="ExternalInput").ap()
    cst = nc.dram_tensor("cst", [128, 8], f32, kind="ExternalInput").ap()
    out = nc.dram_tensor("out", [NP_USED, NT * FW], bf16,
                         kind="ExternalOutput").ap()

    with tile.TileContext(nc) as tc, ExitStack() as ctx:
        cpool = ctx.enter_context(tc.tile_pool(name="consts", bufs=1))
        xpool = ctx.enter_context(tc.tile_pool(name="xin", bufs=NT))
        apool = ctx.enter_context(tc.tile_pool(name="aggps", bufs=2, space="PSUM"))
        qpool = ctx.enter_context(tc.tile_pool(name="polyps", bufs=2, space="PSUM"))
        spool = ctx.enter_context(tc.tile_pool(name="sb", bufs=2))
        ypool = ctx.enter_context(tc.tile_pool(name="yy", bufs=1))

        # --- input loads: FULL 128-partition transfers (the SDMA engine
        # swizzle only sprays a transfer across all 16 engines when the dst
        # spans 128 partitions; partial-partition transfers land on ONE
        # engine at ~26 GB/s). Two HWDGE rings in parallel: sync carries
        # wm+cst+c0+c2, scalar carries c1+c3+dg -> chunks 0 and 1 finish
        # together and early. ---
        xt_t = []
        for j in range(NT):
            xj = xpool.tile([128, NREP * FW], bf16, tag=f"xt{j}", name=f"xt{j}")
            xt_t.append(xj)
        wm_sb = cpool.tile([128, NREP * NP_USED + 128], bf16)
        dg_sb = cpool.tile([128, 8 * NP_USED], bf16)
        cst_sb = cpool.tile([128, 8], f32)
        # The SDMA engines drain concurrent transfers round-robin, so issue
        # order alone does NOT prioritize: everything finishes together.
        # Instead, chunk 2's descriptors are gated on chunks 0+1 completing
        # and chunk 3 on chunk 2 -> pair 0 gets full bandwidth first and
        # compute starts ~6us earlier. wm shares the early window (small,
        # needed first); tiny consts ride the gpsimd ring.
        nc.sync.dma_start(wm_sb[:], wm[:])
        d_c1 = nc.scalar.dma_start(xt_t[1][:], xt[1 * 128:2 * 128, :])
        d_c0 = nc.sync.dma_start(xt_t[0][:], xt[0 * 128:1 * 128, :])
        d_c2 = nc.sync.dma_start(xt_t[2][:], xt[2 * 128:3 * 128, :])
        d_c3 = nc.scalar.dma_start(xt_t[3][:], xt[3 * 128:4 * 128, :])
        add_dep_helper(d_c0.ins, d_c2.ins, sync=True,
                       reason="chunk load priority")
        add_dep_helper(d_c1.ins, d_c2.ins, sync=True,
                       reason="chunk load priority")
        add_dep_helper(d_c2.ins, d_c3.ins, sync=True,
                       reason="chunk load priority")
        nc.gpsimd.dma_start(cst_sb[:], cst[:])
        nc.gpsimd.dma_start(dg_sb[:], dg[:])

        tbias = cst_sb[0:NP_USED, 0:1]
        a0 = cst_sb[0:NP_USED, 1:2]
        dgb = [dg_sb[0:NP_USED, k * NP_USED:(k + 1) * NP_USED]
               for k in range(8)]

        y_t = ypool.tile([NP_USED, NT * FW], bf16, tag="yy", name="yy")

        # --- PE warmup: junk matmuls from a memset tile (no load deps, so
        # they start right after the preamble and keep the HAM clock gate at
        # 8/8 until the first real matmuls arrive ~8.5us later) ---
        warm_sb = cpool.tile([128, FW], bf16)
        nc.vector.memset(warm_sb[:], 0.0)
        wps = apool.tile([NP_USED, 1024], f32, tag="aggps")
        for i in range(32):
            nc.tensor.matmul(wps[:, 0:FW], warm_sb[:, 0:NP_USED],
                             warm_sb[:], start=(i == 0), stop=(i == 31))

        def sbt(name):
            return spool.tile([NP_USED, 2 * FW], bf16, tag=name, name=name)

        # --- agg matmuls for both pairs (PE FIFO: agg p0, agg p1, then
        # poly p0, poly p1 -- poly never blocks agg). Pair-0's t per half
        # right behind its agg group; pair-1's t ops are EMITTED after the
        # pair-0 DVE chain (so the DVE chain's waits aren't coarsened to
        # include them) but still precede pair-0's tanh on the ACT queue. ---
        ps_t = []
        t_tt = []
        for p in range(2):
            ps = wps if p == 0 else apool.tile([NP_USED, 1024], f32,
                                               tag="aggps")
            ps_t.append(ps)
            t_tt.append(sbt("t"))
        for p in range(2):
            ps = ps_t[p]
            for h in range(2):
                j = 2 * p + h
                for r in range(NREP):
                    nc.tensor.matmul(
                        ps[:, h * 512:h * 512 + FW],
                        wm_sb[:, r * NP_USED:(r + 1) * NP_USED],
                        xt_t[j][:, r * FW:(r + 1) * FW],
                        start=(r == 0),
                        stop=(r == NREP - 1),
                    )
                if p == 0:
                    nc.scalar.activation(t_tt[0][:, h * FW:(h + 1) * FW],
                                         ps[:, h * 512:h * 512 + FW],
                                         Act.Identity, bias=tbias)

        # --- per pair: DVE tower + products (9 ops), poly matmuls in DVE
        # completion order, tanh, stores ---
        for p in range(2):
            t_t = t_tt[p]
            t2 = sbt("t2")
            nc.vector.tensor_mul(t2[:], t_t[:], t_t[:])
            u_t = sbt("u")
            nc.vector.tensor_scalar(u_t[:], t2[:], 2.0, -1.0,
                                    op0=Alu.mult, op1=Alu.add)
            u2 = sbt("u2")
            nc.vector.tensor_mul(u2[:], u_t[:], u_t[:])
            v_t = sbt("v")
            nc.vector.tensor_scalar(v_t[:], u2[:], 2.0, -1.0,
                                    op0=Alu.mult, op1=Alu.add)
            ut = sbt("ut")
            nc.vector.tensor_mul(ut[:], u_t[:], t_t[:])
            vt = sbt("vt")
            nc.vector.tensor_mul(vt[:], v_t[:], t_t[:])
            v2 = sbt("v2")
            nc.vector.tensor_mul(v2[:], v_t[:], v_t[:])
            vu = sbt("vu")
            nc.vector.tensor_mul(vu[:], v_t[:], u_t[:])
            uvt = sbt("uvt")
            nc.vector.tensor_mul(uvt[:], u_t[:], vt[:])
            if p == 0:
                # pair-1's t ops: emitted here so they precede pair-0's tanh
                # on the ACT queue but don't contaminate the p0 DVE waits
                for h in range(2):
                    nc.scalar.activation(t_tt[1][:, h * FW:(h + 1) * FW],
                                         ps_t[1][:, h * 512:h * 512 + FW],
                                         Act.Identity, bias=tbias)
            # poly terms ordered by readiness (t at t-copy, tower terms as
            # they complete, products trailing)
            pp = qpool.tile([NP_USED, 1024], f32, tag="polyps")
            terms = [(0, t_t), (1, u_t), (3, v_t), (2, ut),
                     (4, vt), (7, v2), (5, vu), (6, uvt)]
            for i, (k, src) in enumerate(terms):
                for h in range(2):
                    nc.tensor.matmul(pp[:, h * 512:h * 512 + FW], dgb[k],
                                     src[:, h * FW:(h + 1) * FW],
                                     start=(i == 0),
                                     stop=(i == len(terms) - 1))
            # y = tanh(poly + a0) -> bf16 per half; stores ride the sync
            # ring (idle by now), final store covers only 456 columns
            for h in range(2):
                j = 2 * p + h
                nc.scalar.activation(y_t[:, j * FW:(j + 1) * FW],
                                     pp[:, h * 512:h * 512 + FW],
                                     Act.Tanh, bias=a0)
                if p == 1:
                    nc.sync.dma_start(out[:, j * FW:(j + 1) * FW],
                                      y_t[:, j * FW:(j + 1) * FW])
            if p == 0:
                nc.sync.dma_start(out[:, 0:2 * FW], y_t[:, 0:2 * FW])

    nc.compile()
    _PROGRAM = nc
    return nc


LAST_EXEC_NS = None


def _prep_core_input(Xc):
    """[16384, 113] f32 -> [4*128, 9*456] bf16 chunk-major blob.

    Blob row (j*128 + w), col (r*456 + fl) = X row (r*1824 + j*456 + fl),
    channel w (channel rows 113-127 and data rows >= 16384 are zero pad)."""
    XP = np.zeros((NREP * F, NWU), np.float32)
    XP[:RPC] = Xc
    T = (XP.reshape(NREP, NT, FW, NWU).transpose(1, 3, 0, 2)
         .reshape(NT, NWU, NREP * FW))
    blob = np.zeros((NT, 128, NREP * FW), _BF16)
    blob[:, :NWU] = T.astype(_BF16)
    return np.ascontiguousarray(blob.reshape(NT * 128, NREP * FW))


def _unshard_core_output(buf):
    """[117, 4*456] bf16 -> [16384, 13] f32 rows for one core."""
    rows = (np.asarray(buf, np.float32).reshape(NREP, NB, NT, FW)
            .transpose(0, 2, 3, 1).reshape(NREP * F, NB))
    return rows[:RPC]


def kernel(**inputs) -> np.ndarray:
    global LAST_EXEC_NS
    from concourse.bass_utils import run_bass_kernel_spmd

    wm, dg, cst = _fit_tables(inputs)
    X = np.asarray(inputs["wave_features"], np.float32).reshape(B * S, NW)
    X = np.ascontiguousarray(X[:, W0:])

    in_maps = []
    for k in range(NCORES):
        xt_k = _prep_core_input(X[k * RPC:(k + 1) * RPC])
        in_maps.append({"xt": xt_k, "wm": wm, "dg": dg, "cst": cst})

    nc = _build_program()
    trace = os.environ.get("BASS_KERNEL_PROFILE") == "1"
    res = run_bass_kernel_spmd(nc, in_maps, core_ids=list(range(NCORES)),
                               trace=trace)
    LAST_EXEC_NS = res.exec_time_ns
    outs = []
    for k in range(NCORES):
        buf = np.asarray(res.results[k]["out"], np.float32)
        outs.append(_unshard_core_output(buf))
    return np.concatenate(outs, axis=0).reshape(B, S, NB)
